# revision 25
# baseline (speedup 1.0000x reference)
"""EdgeFeatureRGCN Trainium2 kernel: 3-layer RGCN (basis decomposition, mean
aggregation per (dst, relation)) + BatchNorm + LeakyReLU + final L2 normalize.

Strategy (8 NeuronCores, SPMD):
  - Destination-range sharding: core c owns dst nodes [c*NLOC, (c+1)*NLOC).
  - Per-core LPT bin packing permutes dst nodes into 16-dst windows with
    <=256 in-edges each (overflow concentrated in a few 3-chunk windows),
    so nearly every window is exactly two 128-edge chunks.
  - Gathers: one batched dma_gather per <=7-chunk group of consecutive
    windows fetches the 512B src row-PAIR per edge slot (int16 pair ids,
    16-partition index layout), rotated over 4 SWDGE queues so Q7
    descriptor generation runs concurrently.
  - Relation-space selection: per chunk ONE two-scalar DVE op builds
    sel[slot, par*128 + et*16 + col] = 1/cnt(dst,et) via iota==colr
    compare; two matmuls (even/odd row of the gathered pair) accumulate
    acc[feat, 8 rel x 16 dst] in PSUM.  Relation weights W_r = comp @
    bases are folded on host; transform = 8 relation matmuls + root.
  - BN stats (sum, sumsq) via ones-matmul, AllReduce'd; scale/shift +
    leaky ReLU; next-layer node table rebuilt via AllGather into a
    Shared DRAM scratchpad (bf16).
  - Final layer: row L2-normalize, per-core slab output, host unpermute.
"""
import sys
sys.path.insert(0, "/opt/trn_rl_repo")
import numpy as np
import ml_dtypes

from concourse import bass, mybir, bacc, tile
from concourse.masks import make_identity

# problem constants (hardcoded per harness contract)
N, E, R, NB = 50000, 800000, 8, 4
IN, H, OUT = 64, 128, 64
BN_EPS = 1e-5
NCORE = 8
P = 128
WIN = 16                      # dst nodes per selection window
QCAP = 7                      # max chunks per batched dma_gather (ring)
NLOC = N // NCORE             # 6250 real dst nodes per core
NT = (NLOC + P - 1) // P      # 49 transform tiles per core
NLOCP = NT * P                # 6272 padded local slab rows
NWIN = NLOCP // WIN           # 392 windows
NPAD = NCORE * NLOCP          # padded table rows (50176)
SELW = R * WIN                # selection columns per parity (128)
NPAIR = NPAD // 2             # 25088 pair rows (fits int16)
WPT = P // WIN                # 8 windows per 128-dst transform tile
HPT = WPT // 2                # windows per PSUM acc tile (1 bank)
HSPL = 3200                   # slab rows in table block A (25 tiles)

F32 = mybir.dt.float32
BF16 = mybir.dt.bfloat16
I16 = mybir.dt.int16
BF = ml_dtypes.bfloat16
DT = BF16          # on-device data dtype for tables/weights/selection
NPDT = BF


def _row_id(node):
    """global node id -> padded table row id"""
    return (node // NLOC) * NLOCP + (node % NLOC)


def _pack_windows(deg):
    """Pack the core's NLOC dst nodes into NWIN windows of <= WIN slots,
    targeting <= 2*P edges per window (2 chunks); underfilled windows
    last so slab padding stays contiguous at the tail.
    Returns perm (local dst id -> slab row)."""
    import heapq
    CAP = 2 * P
    nfull = NLOC // WIN
    caps = np.zeros(NWIN, np.int64)
    caps[:nfull] = WIN
    rem = NLOC - nfull * WIN
    nbins = nfull + (1 if rem else 0)
    if rem:
        caps[nfull] = rem
    members = [[] for _ in range(NWIN)]
    loads = np.zeros(NWIN, np.int64)
    heap = [(0, b) for b in range(NWIN) if caps[b] > 0]
    heapq.heapify(heap)
    for d in np.argsort(-deg, kind="stable"):
        spill = []
        chosen = None
        while heap:
            _, b = heapq.heappop(heap)
            if len(members[b]) >= caps[b]:
                continue
            if loads[b] + deg[d] <= CAP or chosen is None:
                if loads[b] + deg[d] <= CAP:
                    chosen = b
                    break
                chosen = b  # overfull fallback (lightest)
                break
        b = chosen
        members[b].append(int(d))
        loads[b] += deg[d]
        if len(members[b]) < caps[b]:
            heapq.heappush(heap, (int(loads[b]), b))
        for s in spill:
            heapq.heappush(heap, s)
    # Concentrate the inevitable overflow: allow k windows to take up to
    # 3*P edges (3 chunks) and cap everything else at 2*P, so a ~100-400
    # edge excess costs k extra chunks instead of one per 257-edge window.
    total = int(loads[:nbins].sum())
    k = max(0, -(-max(0, total - CAP * nbins) // P))
    heavy = list(np.argsort(-loads[:nbins])[:k])
    allowed = np.full(nbins, CAP, np.int64)
    for b in heavy:
        allowed[b] = 3 * P
    for _ in range(3):
        over = [b for b in range(nbins) if loads[b] > allowed[b]]
        if not over:
            break
        for o in over:
            for _swap in range(10):
                need = int(loads[o] - allowed[o])
                if need <= 0:
                    break
                # swap one item pair moving as close to `need` as possible
                degs_o = sorted({deg[d] for d in members[o]}, reverse=True)
                by_deg_o = {}
                for d in members[o]:
                    by_deg_o.setdefault(int(deg[d]), d)
                cands = sorted(
                    (b for b in range(nbins)
                     if b != o and allowed[b] - loads[b] > 0),
                    key=lambda b: -(allowed[b] - loads[b]))
                best = None     # (overshoot_rank, o_item, u, u_item)
                for u in cands:
                    slack = int(allowed[u] - loads[u])
                    by_deg_u = {}
                    for d in members[u]:
                        by_deg_u.setdefault(int(deg[d]), d)
                    for do in degs_o:
                        for delta in range(min(need, slack), 0, -1):
                            if do - delta in by_deg_u:
                                cand = (need - delta, by_deg_o[do],
                                        u, by_deg_u[do - delta], delta)
                                if best is None or cand[0] < best[0]:
                                    best = cand
                                break
                    if best is not None and best[0] == 0:
                        break
                if best is None:
                    break
                _, i, u, j, delta = best
                members[o].remove(i)
                members[u].remove(j)
                members[o].append(j)
                members[u].append(i)
                loads[o] -= delta
                loads[u] += delta
    full = sorted(range(nfull), key=lambda b: -loads[b])
    ranked = full + list(range(nfull, NWIN))
    perm = np.zeros(NLOC, np.int64)
    for j, b in enumerate(ranked):
        for i, d in enumerate(members[b]):
            perm[d] = j * WIN + i
    return perm


def _make_groups(quota):
    """Greedy consecutive-window groups with <= QCAP chunks each."""
    groups = []
    w0, cnt = 0, 0
    for w in range(NWIN):
        if cnt + quota[w] > QCAP:
            groups.append((w0, w))
            w0, cnt = w, 0
        cnt += quota[w]
    groups.append((w0, NWIN))
    return groups


def host_prep(edge_index, edge_type):
    """Build per-core gather-index / selection-scalar arrays.

    Returns (quota list [NWIN], totch, per_core list of dicts).
    Per-core arrays:
      idx  [128, totch*8] int16 : src pair-row ids in dma_gather layout
                                  (16-row pattern replicated 8x); group g's
                                  window-flat index i at [i%16, base*8+i//16]
      colr [128, totch]   f32   : par*128 + et*16 + dstcol per slot
      nrm  [128, totch]   f32   : 1/cnt(dst, et) per slot (0 = padding)
    """
    src = np.asarray(edge_index[0], dtype=np.int64)
    dst = np.asarray(edge_index[1], dtype=np.int64)
    et = np.asarray(edge_type, dtype=np.int64)
    seg = dst * R + et
    cnt = np.bincount(seg, minlength=N * R)
    norm = 1.0 / np.maximum(cnt[seg], 1.0)

    core_of = dst // NLOC
    # pass 1: per-core dst load balancing -> slab permutations
    perms = []
    rowid_of_node = np.zeros(N, dtype=np.int64)
    for c in range(NCORE):
        m = core_of == c
        deg = np.bincount(dst[m] - c * NLOC, minlength=NLOC)
        perm = _pack_windows(deg)
        perms.append(perm)
        rowid_of_node[c * NLOC:(c + 1) * NLOC] = c * NLOCP + perm
    # pass 2: per-core edge arrays in permuted slab order
    per_core_edges = []
    nchunks = np.zeros((NCORE, NWIN), dtype=np.int64)
    for c in range(NCORE):
        m = core_of == c
        ldst = perms[c][dst[m] - c * NLOC]
        order = np.argsort(ldst, kind="stable")
        ldst = ldst[order]
        es, ee, en = src[m][order], et[m][order], norm[m][order]
        w = ldst // WIN
        wc = np.bincount(w, minlength=NWIN)
        nchunks[c] = (wc + 127) // 128
        per_core_edges.append((ldst, es, ee, en, w, wc))

    quota = np.maximum(nchunks.max(axis=0), 1)
    base = np.concatenate([[0], np.cumsum(quota)])
    totch = int(base[-1])
    # greedy consecutive-window groups of <= QCAP chunks per dma_gather
    groups = _make_groups(quota)
    # window -> first chunk id of its group
    gb_of_w = np.zeros(NWIN, dtype=np.int64)
    for (w0, w1) in groups:
        gb_of_w[w0:w1] = base[w0]

    per_core = []
    for c in range(NCORE):
        ldst, es, ee, en, w, wc = per_core_edges[c]
        wstart = np.concatenate([[0], np.cumsum(wc)])
        j = np.arange(len(ldst)) - wstart[w]          # rank within window
        g = base[w] + j // 128                        # global chunk id
        s = j % 128                                   # slot within chunk
        colv = ldst - w * WIN                         # dst column in window

        rows = rowid_of_node[es]
        pair = (rows >> 1).astype(np.int16)
        par = (rows & 1).astype(np.int64)

        i_in_g = (g - gb_of_w[w]) * 128 + s           # flat idx within group
        idx16 = np.zeros((16, totch * 8), dtype=np.int16)
        idx16[i_in_g % 16, gb_of_w[w] * 8 + i_in_g // 16] = pair
        idx = np.tile(idx16, (8, 1))                  # replicate to 128 rows

        colr = np.zeros((P, totch), dtype=np.float32)
        colr[s, g] = (par * SELW + ee * WIN + colv).astype(np.float32)
        nrm = np.zeros((P, totch), dtype=np.float32)
        nrm[s, g] = en.astype(np.float32)

        per_core.append({"idx": idx, "colr": colr, "nrm": nrm,
                         "perm": perms[c],
                         "rowid": rowid_of_node[c * NLOC:(c + 1) * NLOC]})
    return [int(q) for q in quota], totch, per_core


def build_program(quota, totch, reps=1, nq=4):
    base = np.concatenate([[0], np.cumsum(quota)])
    groups = _make_groups(quota)
    nc = bacc.Bacc("TRN2", target_bir_lowering=False, debug=False,
                   num_devices=NCORE, num_swdge_queues=nq)

    xtab_d = nc.dram_tensor("xtab", [NPAIR, 2 * IN], DT, kind="ExternalInput")
    xT_d = nc.dram_tensor("xT", [IN, NLOCP], DT, kind="ExternalInput")
    idx_d = nc.dram_tensor("idx", [P, totch * 8], I16, kind="ExternalInput")
    colr_d = nc.dram_tensor("colr", [P, totch], F32, kind="ExternalInput")
    nrm_d = nc.dram_tensor("nrm", [P, totch], F32, kind="ExternalInput")
    iota_d = nc.dram_tensor("iota", [P, 2 * SELW], DT, kind="ExternalInput")
    wl_d = [nc.dram_tensor(f"wl{l}", [P, R * P], DT,
                           kind="ExternalInput") for l in (1, 2, 3)]
    root_d = [nc.dram_tensor(f"root{l}", [P, P], DT, kind="ExternalInput")
              for l in (1, 2, 3)]
    gb_d = [nc.dram_tensor(f"gb{l}", [1, 2 * H], F32, kind="ExternalInput")
            for l in (1, 2)]
    bias3_d = nc.dram_tensor("bias3", [1, OUT], DT, kind="ExternalInput")
    out_d = nc.dram_tensor("out", [NLOCP, OUT], F32, kind="ExternalOutput")

    inW = [IN, H, H]
    outW = [H, H, OUT]
    elemW = [2 * IN, 2 * H, 2 * H]    # gather elem width (pair row)

    with tile.TileContext(nc) as tc:
        with tc.tile_pool(name="sb", bufs=1) as sbp, \
             tc.tile_pool(name="sbl", bufs=4) as sbl, \
             tc.tile_pool(name="sbg", bufs=4) as sbg, \
             tc.tile_pool(name="psA", bufs=2, space="PSUM") as psA, \
             tc.tile_pool(name="psB", bufs=2, space="PSUM") as psB, \
             tc.tile_pool(name="psC", bufs=1, space="PSUM") as psC, \
             tc.tile_pool(name="psS", bufs=1, space="PSUM") as psS, \
             tc.tile_pool(name="dram", bufs=1, space="DRAM") as drp:

            ident = sbp.tile([P, P], DT, tag="ident")
            make_identity(nc, ident[:])
            ones_c = sbp.tile([P, 1], F32, tag="ones_c")
            nc.vector.memset(ones_c[:], 1.0)
            ones_r = sbp.tile([1, P], F32, tag="ones_r")
            nc.vector.memset(ones_r[:], 1.0)
            ones_rb = sbp.tile([1, P], DT, tag="ones_rb")
            nc.vector.memset(ones_rb[:], 1.0)

            iota_t = sbp.tile([P, 2 * SELW], DT, tag="iota")
            nc.sync.dma_start(out=iota_t[:], in_=iota_d[:])
            idx_sb = sbp.tile([P, totch * 8], I16, tag="idx")
            nc.sync.dma_start(out=idx_sb[:], in_=idx_d[:])
            colr_sb = sbp.tile([P, totch], F32, tag="colr")
            nc.sync.dma_start(out=colr_sb[:], in_=colr_d[:])
            nrm_sb = sbp.tile([P, totch], F32, tag="nrm")
            nc.sync.dma_start(out=nrm_sb[:], in_=nrm_d[:])

            wl_sb, root_sb = [], []
            for l in range(3):
                wt = sbp.tile([P, R * P], DT, tag=f"wl{l}")
                nc.sync.dma_start(out=wt[:], in_=wl_d[l][:])
                wl_sb.append(wt)
                rt = sbp.tile([P, P], DT, tag=f"root{l}")
                nc.sync.dma_start(out=rt[:], in_=root_d[l][:])
                root_sb.append(rt)
            gb_sb = []
            for l in range(2):
                gt_ = sbp.tile([1, 2 * H], F32, tag=f"gb{l}")
                nc.sync.dma_start(out=gt_[:], in_=gb_d[l][:])
                gb_sb.append(gt_)
            bias3_sb = sbp.tile([1, OUT], DT, tag="bias3")
            nc.sync.dma_start(out=bias3_sb[:], in_=bias3_d[:])

            KPRE = 96
            selpre = sbp.tile([P, KPRE, 2 * SELW], DT, tag="selpre")
            hTbuf = [sbp.tile([P, NLOCP], DT, tag=f"hT{i}", name=f"hT{i}")
                     for i in range(2)]
            hT = [hTbuf[0], hTbuf[1], hTbuf[0]]
            nc.sync.dma_start(out=hT[0][:IN, :], in_=xT_d[:])
            slab = sbp.tile([P, NT * H], F32, tag="slab")

            allg_in = [drp.tile([NLOCP, P], DT, tag=f"agin{l}", name=f"agin{l}")
                       for l in range(2)]
            tabs_r = [[None,
                       drp.tile([NPAIR, 2 * H], DT, tag=f"tab2_{rr}",
                                name=f"tab2_{rr}", addr_space="Shared"),
                       drp.tile([NPAIR, 2 * H], DT, tag=f"tab3_{rr}",
                                name=f"tab3_{rr}", addr_space="Shared")]
                      for rr in range(reps)]
            st_in = [drp.tile([1, 2 * H], F32, tag=f"sti{l}", name=f"sti{l}")
                     for l in range(2)]
            st_out = [drp.tile([NCORE, 2 * H], F32, tag=f"sto{l}",
                               name=f"sto{l}") for l in range(2)]

            for gp in range(KPRE):
                nc.vector.tensor_scalar(
                    out=selpre[:, gp, :], in0=iota_t[:],
                    scalar1=colr_sb[:, gp:gp + 1],
                    scalar2=nrm_sb[:, gp:gp + 1],
                    op0=mybir.AluOpType.is_equal,
                    op1=mybir.AluOpType.mult)

            for rep in range(reps):
              tabs = tabs_r[rep]
              for l in range(3):
                  last = l == 2
                  iw, ow, ew = inW[l], outW[l], elemW[l]
                  tab_ap = xtab_d[:] if l == 0 else tabs[l][:]
                  if not last:
                      stats = psS.tile([1, 2 * ow], F32, tag="st")
                  acc = None
                  accT = None
                  for gi, (w0, w1) in enumerate(groups):
                      q0 = int(base[w0])
                      qg = int(base[w1] - base[w0])          # chunks in group
                      gt = sbg.tile([P, qg, ew], DT, tag="gt")
                      nc.gpsimd.dma_gather(
                          out_ap=gt[:], in_ap=tab_ap,
                          idxs_ap=idx_sb[:, q0 * 8:(q0 + qg) * 8],
                          num_idxs=qg * P, num_idxs_reg=qg * P,
                          elem_size=ew, queue_num=gi % nq)
                      for w in range(w0, w1):
                          qn = quota[w]
                          if w % HPT == 0:
                              acc = psA.tile([P, HPT, SELW], F32, tag="acc")
                          accw = acc[:iw, w % HPT, :]
                          for q in range(qn):
                              g = int(base[w]) + q
                              qq = g - q0                   # chunk within group
                              if g < KPRE:
                                  selv = selpre[:, g, :]
                              else:
                                  sel = sbl.tile([P, 2 * SELW], DT, tag="sel")
                                  nc.vector.tensor_scalar(
                                      out=sel[:], in0=iota_t[:],
                                      scalar1=colr_sb[:, g:g + 1],
                                      scalar2=nrm_sb[:, g:g + 1],
                                      op0=mybir.AluOpType.is_equal,
                                      op1=mybir.AluOpType.mult)
                                  selv = sel[:]
                              nc.tensor.matmul(
                                  out=accw, lhsT=gt[:, qq, 0:iw],
                                  rhs=selv[:, 0:SELW],
                                  start=(q == 0), stop=False)
                              nc.tensor.matmul(
                                  out=accw, lhsT=gt[:, qq, iw:2 * iw],
                                  rhs=selv[:, SELW:2 * SELW],
                                  start=False, stop=(q == qn - 1))
                          if w % HPT == HPT - 1:
                              if w % WPT == HPT - 1:
                                  accT = sbl.tile([P, R, P], DT, tag="accT")
                              h = (w % WPT) // HPT
                              hw = HPT * WIN
                              nc.scalar.activation(
                                  out=accT[:iw, :, h * hw:(h + 1) * hw]
                                  .rearrange("p r (w n) -> p r w n", w=HPT),
                                  in_=acc[:iw].rearrange(
                                      "p w (r n) -> p r w n", r=R),
                                  func=mybir.ActivationFunctionType.Copy)
                          if w % WPT == WPT - 1:
                              t = w // WPT
                              ot = psB.tile([P, ow], F32, tag="ot")
                              for r in range(R):
                                  nc.tensor.matmul(
                                      out=ot[:], lhsT=accT[:iw, r, :],
                                      rhs=wl_sb[l][:iw, r * P:r * P + ow],
                                      start=(r == 0), stop=False)
                              nc.tensor.matmul(
                                  out=ot[:], lhsT=hT[l][:iw, t * P:(t + 1) * P],
                                  rhs=root_sb[l][:iw, :ow],
                                  start=False, stop=last)
                              if last:
                                  nc.tensor.matmul(
                                      out=ot[:], lhsT=ones_rb[:, :P],
                                      rhs=bias3_sb[:], start=False, stop=True)
                              sl = slab[:, t * ow:(t + 1) * ow]
                              nc.scalar.activation(
                                  out=sl, in_=ot[:],
                                  func=mybir.ActivationFunctionType.Copy)
                              if not last:
                                  sm = sbl.tile([P, 2 * ow], F32, tag="sm")
                                  nc.scalar.activation(
                                      out=sm[:, :ow], in_=ot[:],
                                      func=mybir.ActivationFunctionType.Copy)
                                  nc.vector.tensor_mul(out=sm[:, ow:],
                                                       in0=sl, in1=sl)
                                  kp = P if t < NT - 1 else NLOC - (NT - 1) * P
                                  nc.tensor.matmul(
                                      out=stats[:], lhsT=ones_c[:kp, :],
                                      rhs=sm[:kp, :],
                                      start=(t == 0), stop=(t == NT - 1))
                  if not last:
                      st_sb = sbl.tile([1, 2 * ow], F32, tag="stsb")
                      nc.vector.tensor_copy(out=st_sb[:], in_=stats[:])
                      nc.sync.dma_start(out=st_in[l][:], in_=st_sb[:])
                      nc.gpsimd.collective_compute(
                          "AllGather", mybir.AluOpType.bypass,
                          replica_groups=[list(range(NCORE))],
                          ins=[st_in[l].opt()], outs=[st_out[l].opt()])
                      stg8 = sbl.tile([NCORE, 2 * ow], F32, tag="stg8")
                      nc.sync.dma_start(out=stg8[:], in_=st_out[l][:])
                      stsum = psC.tile([1, 2 * ow], F32, tag="stsum")
                      nc.tensor.matmul(out=stsum[:], lhsT=ones_c[:NCORE, :],
                                       rhs=stg8[:], start=True, stop=True)
                      stg = sbl.tile([1, 2 * ow], F32, tag="stg")
                      nc.vector.tensor_copy(out=stg[:], in_=stsum[:])
                      # scale/shift rows (distinct tiles; avoid slice aliasing)
                      scsh = sbl.tile([1, 2 * ow], F32, tag="scsh")
                      mean_t = sbl.tile([1, ow], F32, tag="bn_mean")
                      tmp = sbl.tile([1, ow], F32, tag="bn_tmp")
                      mean2 = sbl.tile([1, ow], F32, tag="bn_m2")
                      sc_t = sbl.tile([1, ow], F32, tag="bn_sc")
                      ms_t = sbl.tile([1, ow], F32, tag="bn_ms")
                      sh_t = sbl.tile([1, ow], F32, tag="bn_sh")
                      nc.vector.tensor_scalar_mul(out=mean_t[:], in0=stg[:, :ow],
                                                  scalar1=1.0 / N)
                      nc.vector.tensor_scalar_mul(out=tmp[:], in0=stg[:, ow:],
                                                  scalar1=1.0 / N)
                      nc.vector.tensor_mul(out=mean2[:], in0=mean_t[:],
                                           in1=mean_t[:])
                      nc.vector.tensor_sub(out=tmp[:], in0=tmp[:], in1=mean2[:])
                      nc.vector.tensor_scalar_add(out=tmp[:], in0=tmp[:],
                                                  scalar1=BN_EPS)
                      nc.scalar.activation(out=tmp[:], in_=tmp[:],
                                           func=mybir.ActivationFunctionType.Sqrt)
                      nc.vector.reciprocal(out=tmp[:], in_=tmp[:])
                      nc.vector.tensor_mul(out=sc_t[:], in0=tmp[:],
                                           in1=gb_sb[l][:, :ow])
                      nc.vector.tensor_mul(out=ms_t[:], in0=mean_t[:],
                                           in1=sc_t[:])
                      nc.vector.tensor_sub(out=sh_t[:], in0=gb_sb[l][:, ow:],
                                           in1=ms_t[:])
                      nc.vector.tensor_copy(out=scsh[:, :ow], in_=sc_t[:])
                      nc.vector.tensor_copy(out=scsh[:, ow:], in_=sh_t[:])
                      bc = psC.tile([P, 2 * ow], F32, tag="bc")
                      nc.tensor.matmul(out=bc[:], lhsT=ones_r[:, :P],
                                       rhs=scsh[:], start=True, stop=True)
                      bcs = sbl.tile([P, 2 * ow], F32, tag="bcs")
                      nc.vector.tensor_copy(out=bcs[:], in_=bc[:])
                      for t in range(NT):
                          sl = slab[:, t * ow:(t + 1) * ow]
                          nc.vector.tensor_mul(out=sl, in0=sl, in1=bcs[:, :ow])
                          nc.vector.tensor_add(out=sl, in0=sl, in1=bcs[:, ow:])
                          nc.vector.scalar_tensor_tensor(
                              out=sl, in0=sl, scalar=0.1, in1=sl,
                              op0=mybir.AluOpType.mult,
                              op1=mybir.AluOpType.max)
                          hbf = sbl.tile([P, ow], DT, tag="hbf")
                          nc.scalar.activation(
                              out=hbf[:], in_=sl,
                              func=mybir.ActivationFunctionType.Copy)
                          nc.sync.dma_start(
                              out=allg_in[l][t * P:(t + 1) * P, :ow],
                              in_=hbf[:])
                          pt = psC.tile([P, P], DT, tag="pt")
                          nc.tensor.transpose(out=pt[:, :ow]
                                              if ow < P else pt[:],
                                              in_=hbf[:], identity=ident[:])
                          nc.scalar.activation(
                              out=hT[l + 1][:ow, t * P:(t + 1) * P],
                              in_=pt[:ow, :P],
                              func=mybir.ActivationFunctionType.Copy)
                      nc.gpsimd.collective_compute(
                          "AllGather", mybir.AluOpType.bypass,
                          replica_groups=[list(range(NCORE))],
                          ins=[allg_in[l].opt()], outs=[tabs[l + 1].opt()])
                  else:
                      for t in range(NT):
                          sl = slab[:, t * ow:(t + 1) * ow]
                          sq = sbl.tile([P, ow], F32, tag="sqf")
                          rs = sbl.tile([P, 1], F32, tag="rs")
                          nc.scalar.activation(
                              out=sq[:], in_=sl,
                              func=mybir.ActivationFunctionType.Square,
                              accum_out=rs[:])
                          nc.scalar.activation(
                              out=rs[:], in_=rs[:],
                              func=mybir.ActivationFunctionType.Sqrt)
                          nc.vector.tensor_scalar_max(out=rs[:], in0=rs[:],
                                                      scalar1=1e-12)
                          nc.vector.reciprocal(out=rs[:], in_=rs[:])
                          fin = sbl.tile([P, ow], F32, tag="fin")
                          nc.vector.tensor_tensor(
                              out=fin[:], in0=sl,
                              in1=rs[:].to_broadcast([P, ow]),
                              op=mybir.AluOpType.mult)
                          nc.sync.dma_start(
                              out=out_d[t * P:(t + 1) * P, :], in_=fin[:])
    nc.compile()
    return nc


def make_inputs(inputs, quota, totch, per_core):
    """Build per-core in_maps from the reference inputs."""
    x = np.asarray(inputs["x"], np.float32)
    xtab = np.zeros((NPAD, IN), dtype=NPDT)
    for c in range(NCORE):
        xtab[per_core[c]["rowid"]] = x[c * NLOC:(c + 1) * NLOC]
    xtab = xtab.reshape(NPAIR, 2 * IN)
    iota = np.broadcast_to(np.arange(2 * SELW, dtype=np.float32),
                           (P, 2 * SELW)).astype(NPDT)
    wts = {"iota": iota}
    for l, (iw, ow) in enumerate(((IN, H), (H, H), (H, OUT))):
        comp = np.asarray(inputs[f"comp{l + 1}"], np.float32)    # [R, NB]
        bas = np.asarray(inputs[f"bases{l + 1}"], np.float32)    # [NB, iw, ow]
        W = np.einsum("rb,bio->rio", comp, bas)                  # [R, iw, ow]
        wl = np.zeros((P, R * P), dtype=NPDT)
        for r in range(R):
            wl[:iw, r * P:r * P + ow] = W[r]
        wts[f"wl{l + 1}"] = wl
        rt = np.zeros((P, P), dtype=NPDT)
        rt[:iw, :ow] = np.asarray(inputs[f"root{l + 1}"], np.float32)
        wts[f"root{l + 1}"] = rt
    for l in (1, 2):
        wts[f"gb{l}"] = np.concatenate(
            [np.asarray(inputs[f"g{l}"], np.float32),
             np.asarray(inputs[f"b{l}"], np.float32)])[None, :]
    wts["bias3"] = np.asarray(inputs["bias3"], np.float32).astype(NPDT)[None, :]

    in_maps = []
    for c in range(NCORE):
        xloc = np.zeros((NLOCP, IN), np.float32)
        xloc[per_core[c]["perm"]] = x[c * NLOC:(c + 1) * NLOC]
        m = {"xtab": xtab,
             "xT": np.ascontiguousarray(xloc.T).astype(NPDT),
             "idx": per_core[c]["idx"],
             "colr": per_core[c]["colr"],
             "nrm": per_core[c]["nrm"]}
        m.update(wts)
        in_maps.append(m)
    return in_maps


_CACHE = {}


def kernel(**inputs) -> np.ndarray:
    quota, totch, per_core = host_prep(inputs["edge_index"],
                                       inputs["edge_type"])
    key = tuple(quota)
    if key not in _CACHE:
        _CACHE[key] = build_program(quota, totch)
    nc = _CACHE[key]
    from concourse.bass2jax import run_bass_via_pjrt
    in_maps = make_inputs(inputs, quota, totch, per_core)
    res = run_bass_via_pjrt(nc, in_maps, n_cores=NCORE)
    out = np.concatenate(
        [res[c]["out"][per_core[c]["perm"]] for c in range(NCORE)], axis=0)
    return out.astype(np.float32)

